# revision 47
# baseline (speedup 1.0000x reference)
"""BC6H surrogate block-level decode kernel for 8 Trainium2 NeuronCores.

Full-input contract: kernel(**inputs) takes the complete arrays from
setup_inputs() and returns the full (3, 4096, 4096) image.  The block
dimension (nb = 1048576) is sharded 8 ways (pure data parallel); each core
runs an identical Bass/Tile program on its 131072-block shard.

Math (per 4x4 block b, pixel p in 0..15, channel c in 0..2):
  sig_e = sigmoid(endpoints)                      (4 endpoints x 3 ch)
  w     = (63*sig(idx) + clip(7*sig(idx)-3,0,1))/64      exact LUT lerp
  m     = softmax(logits) @ bank                  (soft partition mask)
  e_u_i = 31248*sig_e_i + 248                     (uf16-domain endpoints)
  u     = (m*(e0(1-w)+e1 w) + (1-m)*(e2(1-w)+e3 w)) / 1024
  hh    = round(u - 1.5009765625) ;  out = (u - hh) * 2^(hh-14)

fp16 implementation (numerically validated: rel err ~2.7e-3 « 2e-2 gate):
  * all SBUF tiles fp16 except the final output tile (fp32); DVE ALUs
    compute at fp32 internally so the magic-round trick uses fp32 MAGIC.
  * (c, p, g) layout with g (blocks-per-partition-row) innermost, so every
    elementwise operand is packed (innermost stride 1) -> DVE 2x fp16 mode;
    coefficient broadcasts use stride-0 middle dims.
  * mask path: PE-transpose raw fp16 logits (128x128 chunks) -> PSUM, ACT
    exp -> SBUF transposed, per chunk one K=128 matmul with a block-diagonal
    (p,q)-interleaved bank producing num[(p, gg)] + den in fp32 PSUM; ACT
    copies PSUM -> fp16 num_t in (p, g) layout (2-free-dim PSUM APs only).
  * decode: hh = TS(u', +MAGIC, -MAGIC) (fp32 magic round), ACT exp gives
    2^(hh-14), sdf = u' - hh, out = (sdf + FLOOR_OFF_H)*e2 in one fused
    2-src custom DVE op that also converts to fp32 and scatters to the
    block-major (g, c, p) output layout for contiguous 12KB DMA descriptors.
  * decode tail (exp + final fuse + store) is software-pipelined one
    supertile behind the front end so the ACT queue never stalls the
    vector queue across iterations.
"""

import sys

sys.path.insert(0, "/opt/trn_rl_repo")

from contextlib import ExitStack

import numpy as np

import concourse.bass as bass
import concourse.tile as tile
from concourse import bacc, mybir
from concourse import bass_utils
from concourse import dve_ops
from concourse.dve_ops import DveOp
from concourse.dve_spec import (
    Spec,
    Src0,
    Src1,
    C0,
    C1,
    C2,
    One,
    relu,
    minn,
    lower,
    _has_src1,
)
from concourse.dve_uop import DveOpSpec

F32 = mybir.dt.float32
F16 = mybir.dt.float16
AOp = mybir.AluOpType
AFn = mybir.ActivationFunctionType

# ---------------------------------------------------------------- constants
NB = 1048576
N_CORES = 8
NB_CORE = NB // N_CORES            # 131072 blocks per core
G = 64                             # blocks per partition-row per supertile
H = W = 4096
BY = BX = 1024

EU_SCALE = 31248.0 / 1024.0        # 30.515625 (exact in fp16)
EU_BIAS = 248.0 / 1024.0
FLOOR_OFF_H = 1.5009765625         # u' = u - FLOOR_OFF_H; hh = round(u')
MAGIC = 12582912.0                 # 1.5 * 2^23 (fp32 magic round)
LN2 = 0.6931471805599453

# ------------------------------------------------------- custom DVE ops
_REGISTERED = {}


def _register(name, spec):
    if name in _REGISTERED:
        return _REGISTERED[name]
    if name not in dve_ops._SUB_OPCODE_FOR_NAME:
        row = max(dve_ops._SUB_OPCODE_FOR_NAME.values()) + 1
        assert row < 0x20, "custom-DVE opcode rows exhausted"
        dve_ops._SUB_OPCODE_FOR_NAME[name] = row
    row = dve_ops._SUB_OPCODE_FOR_NAME[name]
    shas = {}
    for ver in ("v3", "v4"):
        try:
            uops = lower(spec, ver=ver)
            shas[ver] = DveOpSpec(
                name=name, opcode=row, uops=uops, rd1_en=_has_src1(spec)
            ).sha(ver)
        except Exception:
            if ver == "v3":
                raise
    op = DveOp(name, spec, subdim=False, uops_sha=shas)
    dve_ops.OPS.append(op)
    dve_ops.CUSTOM_DVE_SPECS[name] = op.spec
    _REGISTERED[name] = op
    return op


# w = s - (s - min(relu(s*c0 + c1), 1))*c2 = (63 s + clip(7s-3, 0, 1)) / 64
BC6W = _register(
    "BC6W_ANT",
    Spec(
        body=Src0 - (Src0 - minn(relu(Src0 * C0 + C1), One)) * C2,
        reference=lambda in0, in1, c0, c1, c2: (
            in0.astype(np.float32)
            - (
                in0.astype(np.float32)
                - np.minimum(
                    np.maximum(in0.astype(np.float32) * c0 + c1, 0.0), 1.0
                )
            )
            * c2
        ).astype(np.float32),
    ),
)

# (a - b) * c0   (endpoint-difference coefficients)
CDIFFS = _register(
    "BC6CDIFFS_ANT",
    Spec(
        body=(Src0 - Src1) * C0,
        reference=lambda in0, in1, c0, c1, c2: (
            (in0.astype(np.float32) - in1.astype(np.float32)) * np.float32(c0)
        ).astype(np.float32),
    ),
)

# (a * b) * c0   (fold 1/den + EU scale into coefficients)
CMULS = _register(
    "BC6CMULS_ANT",
    Spec(
        body=(Src0 * Src1) * C0,
        reference=lambda in0, in1, c0, c1, c2: (
            (in0.astype(np.float32) * in1.astype(np.float32)) * np.float32(c0)
        ).astype(np.float32),
    ),
)

# (a + c0) * b * c1  (final decode fuse: (sdf + FLOOR_OFF_H) * e2 * K,
# K corrects the fp16 rounding of the ACT exp bias)
FMADD = _register(
    "BC6FMADDS_ANT",
    Spec(
        body=(Src0 + C0) * Src1 * C1,
        reference=lambda in0, in1, c0, c1, c2: (
            (in0.astype(np.float32) + np.float32(c0))
            * in1.astype(np.float32)
            * np.float32(c1)
        ).astype(np.float32),
    ),
)

# fp32 value the ACT bias tile holds, and the exact correction factor
# K = 2^-14 / exp(bias32) folded into the final fused multiply
_EXP_BIAS = float(np.float32(-14.0 * LN2))
_EXP_K = float(np.exp(-(14.0 * LN2 + _EXP_BIAS)))


# ------------------------------------------------------- bass kernel build
def _ap(base, dims):
    """Manual free-dim AP: keep base's partition dim, set free dims."""
    return bass.AP(base.tensor, base.offset, [list(base.ap[0])] + dims)


def build_kernel(nb_core=NB_CORE, g=G, eng_cfg=None):
    st_blocks = 128 * g
    n_st = nb_core // st_blocks
    assert nb_core % st_blocks == 0
    n_ch = (g * 32) // 128          # 128-col logit chunks per supertile
    assert n_ch % 4 == 0
    n_pt = n_ch // 4                # PSUM matmul tiles (4 chunks each)

    nc = bacc.Bacc(
        "TRN2",
        target_bir_lowering=False,
        debug=False,
        enable_asserts=False,
        num_devices=1,
    )

    # which engine runs each elementwise op (V=vector, G=gpsimd)
    cfg = dict(
        d02="V", d13="V", dd="V", bu="V", cur="V", dur="V",
        A1="V", A2="V", A3="V", A4="V", A5="V", A6="V",
        T1="V", T3="V", FM="V",
    )
    if eng_cfg:
        cfg.update(eng_cfg)

    ep = nc.dram_tensor("endpoints", [nb_core, 12], F16, kind="ExternalInput").ap()
    ix = nc.dram_tensor("indices", [nb_core, 16], F16, kind="ExternalInput").ap()
    lg = nc.dram_tensor("logits", [nb_core, 32], F16, kind="ExternalInput").ap()
    # bank_diag [128, 68]: row k (band q = k//32) has bank3[k%32, p] at col
    # 4p+q and 1.0 at col 64+q ("p=16"); zeros elsewhere.  One K=128 matmul
    # per transposed chunk yields num[(p,q)] + den interleaved, uniform
    # stride 4 in p, so the ACT PSUM->SBUF copy and all consumers use
    # affine APs.
    bank = nc.dram_tensor("bankd", [128, 68], F16, kind="ExternalInput").ap()
    ident = nc.dram_tensor("ident", [128, 128], F16, kind="ExternalInput").ap()
    out = nc.dram_tensor("out", [nb_core, 48], F32, kind="ExternalOutput").ap()

    with nc.allow_low_precision("fp16 kernel by design (validated 2.7e-3)"):
        with tile.TileContext(nc) as tc, ExitStack() as ctx:
            const_pool = ctx.enter_context(tc.tile_pool(name="const", bufs=1))
            in_pool = ctx.enter_context(tc.tile_pool(name="inp", bufs=4))
            sig_pool = ctx.enter_context(tc.tile_pool(name="sig", bufs=4))
            msk_pool = ctx.enter_context(tc.tile_pool(name="msk", bufs=3))
            et_pool = ctx.enter_context(tc.tile_pool(name="et", bufs=2))
            ta_pool = ctx.enter_context(tc.tile_pool(name="ta", bufs=2))
            cf_pool = ctx.enter_context(tc.tile_pool(name="cf", bufs=3))
            big_pool = ctx.enter_context(tc.tile_pool(name="big", bufs=3))
            dec_pool = ctx.enter_context(tc.tile_pool(name="dec", bufs=3))
            out_pool = ctx.enter_context(tc.tile_pool(name="outp", bufs=2))
            ps_t = ctx.enter_context(tc.tile_pool(name="ps_t", bufs=2, space="PSUM"))
            ps_mm = ctx.enter_context(tc.tile_pool(name="ps_mm", bufs=4, space="PSUM"))

            bank_t = const_pool.tile([128, 68], F16)
            nc.sync.dma_start(bank_t[:], bank)
            id_t = const_pool.tile([128, 128], F16)
            nc.sync.dma_start(id_t[:], ident)
            ebias_t = const_pool.tile([128, 1], F32)
            nc.gpsimd.memset(ebias_t[:], _EXP_BIAS)

            eng = {"V": nc.vector, "G": nc.gpsimd}

            def stage_in(t):
                """Loads, sigmoids, w, endpoint coeffs, mask path -> num_t."""
                b0 = t * st_blocks
                # ep and ix share one tile so a single ACT sigmoid covers both
                epix_t = in_pool.tile([128, g * 28], F16, tag="epix")
                ep_t = epix_t[:, 0 : g * 12]
                ix_t = epix_t[:, g * 12 : g * 28]
                lg_t = in_pool.tile([128, g * 32], F16, tag="lg")
                with tc.high_priority(offset=200):
                    nc.sync.dma_start(
                        ep_t,
                        ep[b0 : b0 + st_blocks, :].rearrange(
                            "(r g) d -> r (g d)", g=g
                        ),
                    )
                    nc.sync.dma_start(
                        ix_t,
                        ix[b0 : b0 + st_blocks, :].rearrange(
                            "(r g) d -> r (g d)", g=g
                        ),
                    )
                    nc.sync.dma_start(
                        lg_t[:],
                        lg[b0 : b0 + st_blocks, :].rearrange(
                            "(r g) d -> r (g d)", g=g
                        ),
                    )

                epix_s = sig_pool.tile([128, g * 28], F16, tag="epixs")
                # pull the sigmoid ahead of the rest of this iteration's ACT
                # work so w/d02 never wait on it
                with tc.high_priority(offset=140):
                    nc.scalar.activation(epix_s[:], epix_t[:], AFn.Sigmoid)

                s2u = cf_pool.tile([128, g * 3], F16, tag="s2u")
                nc.scalar.activation(
                    _ap(s2u[:], [[g, 3], [1, g]]),
                    _ap(epix_s[:, 6:9], [[1, 3], [12, g]]), AFn.Copy,
                    bias=float(EU_BIAS - FLOOR_OFF_H), scale=float(EU_SCALE),
                )

                # ---- logits: PE transpose -> ACT exp -> E_T (fp16) ----
                e_T = et_pool.tile([128, g * 32], F16, tag="eT")
                for j in range(0, n_ch, 8):
                    pst = ps_t.tile([128, 1024], F16, tag="pst")
                    for q in range(8):
                        ch = j + q
                        nc.tensor.transpose(
                            pst[:, 128 * q : 128 * (q + 1)],
                            lg_t[:, 128 * ch : 128 * (ch + 1)],
                            id_t[:],
                        )
                    nc.scalar.activation(
                        e_T[:, 128 * j : 128 * (j + 8)], pst[:], AFn.Exp
                    )

                # ---- per-chunk matmuls + ACT copies PSUM -> num_t ----
                # num_t [128, 17*g]: col p*g + gg, p=16 row holds den.
                num_t = msk_pool.tile([128, 17 * g], F16, tag="numt")
                for tp in range(n_pt):
                    pmm = ps_mm.tile([128, 272], F32, tag="pmm")
                    for chp in range(4):
                        ch = 4 * tp + chp
                        nc.tensor.matmul(
                            pmm[:, 68 * chp : 68 * (chp + 1)],
                            e_T[:, 128 * ch : 128 * (ch + 1)],
                            bank_t[:],
                            start=True,
                            stop=True,
                        )
                    nc.scalar.activation(
                        _ap(num_t[:, 16 * tp : 16 * tp + 4], [[4, 4], [1, 4], [g, 17]]),
                        _ap(pmm[:], [[68, 4], [1, 4], [4, 17]]),
                        AFn.Copy,
                    )

                return dict(b0=b0, epix_s=epix_s, num_t=num_t, s2u=s2u)

            def stage_in_back(c):
                """V/G input-side elementwise ops, emitted after the previous
                supertile's assembly so the vector queue never waits on this
                iteration's ACT sigmoid."""
                epix_s = c["epix_s"]

                def eslice(i):
                    return _ap(epix_s[:, 3 * i : 3 * i + 3], [[1, 3], [12, g]])

                # ---- w (custom DVE, (p,g)-transposing write) ----
                w_t = sig_pool.tile([128, g * 16], F16, tag="w")
                nc.vector._custom_dve(
                    BC6W,
                    out=w_t[:],
                    in0=_ap(epix_s[:, g * 12 : g * 12 + 16], [[1, 16], [16, g]]),
                    s0=7.0,
                    s1=-3.0,
                    imm2=1.0 / 64.0,
                )

                # ---- endpoint coefficients, (c, g) layout ----
                d02 = cf_pool.tile([128, g * 3], F16, tag="d02")
                d02v = _ap(d02[:], [[g, 3], [1, g]])
                eng[cfg["d02"]].tensor_sub(d02v, eslice(0), eslice(2))
                d13 = cf_pool.tile([128, g * 3], F16, tag="d13")
                d13v = _ap(d13[:], [[g, 3], [1, g]])
                eng[cfg["d13"]].tensor_sub(d13v, eslice(1), eslice(3))
                dd = cf_pool.tile([128, g * 3], F16, tag="dd")
                ddv = _ap(dd[:], [[g, 3], [1, g]])
                eng[cfg["dd"]].tensor_sub(ddv, d13v, d02v)
                bu = cf_pool.tile([128, g * 3], F16, tag="bu")
                nc.vector._custom_dve(
                    CDIFFS,
                    out=_ap(bu[:], [[g, 3], [1, g]]),
                    in0=eslice(3),
                    in1=eslice(2),
                    s0=EU_SCALE,
                )
                # Bu*w: consumed by A5 only next iteration.
                w_b = _ap(w_t[:], [[0, 3], [g, 16], [1, g]])
                tB = big_pool.tile([128, g * 48], F16, tag="tB")
                eng[cfg["A4"]].tensor_mul(
                    tB[:], _ap(bu[:], [[g, 3], [0, 16], [1, g]]), w_b
                )
                c.update(w_t=w_t, d02v=d02v, ddv=ddv, tB=tB)

            def stage_asm(c):
                tB = c["tB"]
                """rcp-folded coefficients + assembly + round (V/G)."""
                rcp = cf_pool.tile([128, g], F16, tag="rcp")
                nc.vector.reciprocal(rcp[:], c["num_t"][:, 16 * g : 17 * g])
                rcp_b = _ap(rcp[:], [[0, 3], [1, g]])
                cur = cf_pool.tile([128, g * 3], F16, tag="cur")
                nc.vector._custom_dve(
                    CMULS, out=_ap(cur[:], [[g, 3], [1, g]]),
                    in0=c["d02v"], in1=rcp_b, s0=EU_SCALE,
                )
                dur = cf_pool.tile([128, g * 3], F16, tag="dur")
                nc.vector._custom_dve(
                    CMULS, out=_ap(dur[:], [[g, 3], [1, g]]),
                    in0=c["ddv"], in1=rcp_b, s0=EU_SCALE,
                )

                def cb(tile_):   # (c,g) coeff -> (c, p, g) broadcast
                    return _ap(tile_[:], [[g, 3], [0, 16], [1, g]])

                w_b = _ap(c["w_t"][:], [[0, 3], [g, 16], [1, g]])
                num_b = _ap(c["num_t"][:], [[0, 3], [g, 16], [1, g]])

                tA = ta_pool.tile([128, g * 48], F16, tag="tA")
                up = big_pool.tile([128, g * 48], F16, tag="up")
                hh = big_pool.tile([128, g * 48], F16, tag="hh")

                eng[cfg["A1"]].tensor_mul(tA[:], cb(dur), w_b)       # Dur*w
                eng[cfg["A2"]].tensor_add(tA[:], tA[:], cb(cur))     # +Cur
                eng[cfg["A3"]].tensor_mul(tA[:], tA[:], num_b)       # *num
                eng[cfg["A5"]].tensor_add(tA[:], tA[:], tB[:])       # +Bu*w
                eng[cfg["A6"]].tensor_add(up[:], tA[:], cb(c["s2u"]))  # = u'
                # hh = round(u') via fp32 magic add (DVE ALU is fp32)
                eng[cfg["T1"]].tensor_scalar(
                    hh[:], up[:], MAGIC, MAGIC, AOp.add, AOp.subtract
                )
                return dict(b0=c["b0"], up=up, hh=hh)

            def tail_sub(a):
                """sdf = u' - hh; deps are one iteration old."""
                sdf = dec_pool.tile([128, g * 48], F16, tag="sdf")
                eng[cfg["T3"]].tensor_sub(sdf[:], a["up"][:], a["hh"][:])
                return sdf

            def tail_exp(a):
                """ACT exp of hh, emitted last in the iteration so the ACT
                queue serves this iteration's sigmoid first (e2 is consumed
                only next iteration)."""
                e2 = dec_pool.tile([128, g * 48], F16, tag="e2")
                nc.scalar.activation(
                    e2[:], a["hh"][:], AFn.Exp, bias=ebias_t[:], scale=LN2
                )
                return e2

            def tail_store(a, e2, sdf):
                o_t = out_pool.tile([128, g * 48], F32, tag="o")
                # out is (g, c, p) block-major for contiguous DMA; the fused
                # op reads (c, p, g) streams ((c,p) collapses to one uniform
                # dim) and scatters on write (1x op).
                nc.vector._custom_dve(
                    FMADD,
                    out=_ap(o_t[:], [[1, 48], [48, g]]),
                    in0=_ap(sdf[:], [[g, 48], [1, g]]),
                    in1=_ap(e2[:], [[g, 48], [1, g]]),
                    s0=FLOOR_OFF_H,
                    s1=_EXP_K,
                )
                nc.sync.dma_start(
                    out[a["b0"] : a["b0"] + st_blocks, :].rearrange(
                        "(r g) d -> r (g d)", g=g
                    ),
                    o_t[:],
                )

            # 3-stage software pipeline: mask path of tile t runs alongside
            # the assembly of t-1, the exp/subtract of t-2, and the final
            # fuse+store of t-3, so no engine queue ever waits on a
            # same-iteration cross-engine product (gpsimd's slow T3 gets a
            # full iteration before FM consumes it).
            in_ctxs = {}
            asm_outs = {}
            sub_outs = {}
            exp_outs = {}
            for t in range(n_st + 3):
                if 2 <= t < n_st + 2:
                    with tc.high_priority(offset=70):
                        sub_outs[t - 2] = tail_sub(asm_outs[t - 2])
                if t < n_st:
                    in_ctxs[t] = stage_in(t)
                if 1 <= t <= n_st:
                    asm_outs[t - 1] = stage_asm(in_ctxs.pop(t - 1))
                if t < n_st:
                    with tc.high_priority(offset=-15):
                        stage_in_back(in_ctxs[t])
                if 2 <= t < n_st + 2:
                    exp_outs[t - 2] = tail_exp(asm_outs[t - 2])
                if t >= 3:
                    with tc.high_priority(offset=70):
                        tail_store(
                            asm_outs.pop(t - 3),
                            exp_outs.pop(t - 3),
                            sub_outs.pop(t - 3),
                        )

    nc.compile()
    return nc


# ------------------------------------------------------- host-side driver
_NC_CACHE = {}


def _get_nc():
    if "nc" not in _NC_CACHE:
        _NC_CACHE["nc"] = build_kernel()
    return _NC_CACHE["nc"]


def make_in_maps(endpoints, indices, partition_logits, partition_bank, nb=NB):
    """Shard + pack host inputs into the 8 per-core input dicts."""
    bank16 = partition_bank.astype(np.float16)       # 0/1 exact
    bankd = np.zeros((128, 68), dtype=np.float16)
    for q in range(4):
        rows = slice(32 * q, 32 * (q + 1))
        bankd[rows, q : 64 + q : 4] = bank16         # col 4p+q, p=0..15
        bankd[rows, 64 + q] = 1.0                    # den
    ident = np.eye(128, dtype=np.float16)

    ep_flat = endpoints.reshape(nb, 12).astype(np.float16)
    ixf = indices.astype(np.float16)
    lgf = partition_logits.astype(np.float16)
    nbc = nb // N_CORES
    in_maps = []
    for c in range(N_CORES):
        sl = slice(c * nbc, (c + 1) * nbc)
        in_maps.append(
            {
                "endpoints": np.ascontiguousarray(ep_flat[sl]),
                "indices": np.ascontiguousarray(ixf[sl]),
                "logits": np.ascontiguousarray(lgf[sl]),
                "bankd": bankd,
                "ident": ident,
            }
        )
    return in_maps


def blocks_to_img(blocks):
    """[NB, 48] c-major blocks -> (3, H, W) image."""
    return (
        blocks.reshape(BY, BX, 3, 4, 4)
        .transpose(2, 0, 3, 1, 4)
        .reshape(3, H, W)
        .astype(np.float32)
    )


def kernel(endpoints, indices, partition_logits, partition_bank, weight_lut):
    endpoints = np.asarray(endpoints, dtype=np.float32)
    indices = np.asarray(indices, dtype=np.float32)
    partition_logits = np.asarray(partition_logits, dtype=np.float32)
    partition_bank = np.asarray(partition_bank, dtype=np.float32)
    assert endpoints.shape[0] == NB

    in_maps = make_in_maps(endpoints, indices, partition_logits, partition_bank)
    nc = _get_nc()
    res = bass_utils.run_bass_kernel_spmd(
        nc, in_maps, core_ids=list(range(N_CORES))
    )
    blocks = np.concatenate(
        [res.results[c]["out"] for c in range(N_CORES)], axis=0
    )
    return blocks_to_img(blocks)


# revision 48
# speedup vs baseline: 1.0634x; 1.0634x over previous
"""BC6H surrogate block-level decode kernel for 8 Trainium2 NeuronCores.

Full-input contract: kernel(**inputs) takes the complete arrays from
setup_inputs() and returns the full (3, 4096, 4096) image.  The block
dimension (nb = 1048576) is sharded 8 ways (pure data parallel); each core
runs an identical Bass/Tile program on its 131072-block shard.

Math (per 4x4 block b, pixel p in 0..15, channel c in 0..2):
  sig_e = sigmoid(endpoints)                      (4 endpoints x 3 ch)
  w     = (63*sig(idx) + clip(7*sig(idx)-3,0,1))/64      exact LUT lerp
  m     = softmax(logits) @ bank                  (soft partition mask)
  e_u_i = 31248*sig_e_i + 248                     (uf16-domain endpoints)
  u     = (m*(e0(1-w)+e1 w) + (1-m)*(e2(1-w)+e3 w)) / 1024
  hh    = round(u - 1.5009765625) ;  out = (u - hh) * 2^(hh-14)

fp16 implementation (numerically validated: rel err ~2.7e-3 « 2e-2 gate):
  * all SBUF tiles fp16 except the final output tile (fp32); DVE ALUs
    compute at fp32 internally so the magic-round trick uses fp32 MAGIC.
  * (c, p, g) layout with g (blocks-per-partition-row) innermost, so every
    elementwise operand is packed (innermost stride 1) -> DVE 2x fp16 mode;
    coefficient broadcasts use stride-0 middle dims.
  * mask path: PE-transpose raw fp16 logits (128x128 chunks) -> PSUM, ACT
    exp -> SBUF transposed, per chunk one K=128 matmul with a block-diagonal
    (p,q)-interleaved bank producing num[(p, gg)] + den in fp32 PSUM; ACT
    copies PSUM -> fp16 num_t in (p, g) layout (2-free-dim PSUM APs only).
  * decode: hh = TS(u', +MAGIC, -MAGIC) (fp32 magic round), ACT exp gives
    2^(hh-14), sdf = u' - hh, out = (sdf + FLOOR_OFF_H)*e2 in one fused
    2-src custom DVE op that also converts to fp32 and scatters to the
    block-major (g, c, p) output layout for contiguous 12KB DMA descriptors.
  * decode tail (exp + final fuse + store) is software-pipelined one
    supertile behind the front end so the ACT queue never stalls the
    vector queue across iterations.
"""

import sys

sys.path.insert(0, "/opt/trn_rl_repo")

from contextlib import ExitStack

import numpy as np

import concourse.bass as bass
import concourse.tile as tile
from concourse import bacc, mybir
from concourse import bass_utils
from concourse import dve_ops
from concourse.dve_ops import DveOp
from concourse.dve_spec import (
    Spec,
    Src0,
    Src1,
    C0,
    C1,
    C2,
    One,
    relu,
    minn,
    lower,
    _has_src1,
)
from concourse.dve_uop import DveOpSpec

F32 = mybir.dt.float32
F16 = mybir.dt.float16
AOp = mybir.AluOpType
AFn = mybir.ActivationFunctionType

# ---------------------------------------------------------------- constants
NB = 1048576
N_CORES = 8
NB_CORE = NB // N_CORES            # 131072 blocks per core
G = 64                             # blocks per partition-row per supertile
H = W = 4096
BY = BX = 1024

EU_SCALE = 31248.0 / 1024.0        # 30.515625 (exact in fp16)
EU_BIAS = 248.0 / 1024.0
FLOOR_OFF_H = 1.5009765625         # u' = u - FLOOR_OFF_H; hh = round(u')
MAGIC = 12582912.0                 # 1.5 * 2^23 (fp32 magic round)
LN2 = 0.6931471805599453

# ------------------------------------------------------- custom DVE ops
_REGISTERED = {}


def _register(name, spec):
    if name in _REGISTERED:
        return _REGISTERED[name]
    if name not in dve_ops._SUB_OPCODE_FOR_NAME:
        row = max(dve_ops._SUB_OPCODE_FOR_NAME.values()) + 1
        assert row < 0x20, "custom-DVE opcode rows exhausted"
        dve_ops._SUB_OPCODE_FOR_NAME[name] = row
    row = dve_ops._SUB_OPCODE_FOR_NAME[name]
    shas = {}
    for ver in ("v3", "v4"):
        try:
            uops = lower(spec, ver=ver)
            shas[ver] = DveOpSpec(
                name=name, opcode=row, uops=uops, rd1_en=_has_src1(spec)
            ).sha(ver)
        except Exception:
            if ver == "v3":
                raise
    op = DveOp(name, spec, subdim=False, uops_sha=shas)
    dve_ops.OPS.append(op)
    dve_ops.CUSTOM_DVE_SPECS[name] = op.spec
    _REGISTERED[name] = op
    return op


# w = s - (s - min(relu(s*c0 + c1), 1))*c2 = (63 s + clip(7s-3, 0, 1)) / 64
BC6W = _register(
    "BC6W_ANT",
    Spec(
        body=Src0 - (Src0 - minn(relu(Src0 * C0 + C1), One)) * C2,
        reference=lambda in0, in1, c0, c1, c2: (
            in0.astype(np.float32)
            - (
                in0.astype(np.float32)
                - np.minimum(
                    np.maximum(in0.astype(np.float32) * c0 + c1, 0.0), 1.0
                )
            )
            * c2
        ).astype(np.float32),
    ),
)

# (a - b) * c0   (endpoint-difference coefficients)
CDIFFS = _register(
    "BC6CDIFFS_ANT",
    Spec(
        body=(Src0 - Src1) * C0,
        reference=lambda in0, in1, c0, c1, c2: (
            (in0.astype(np.float32) - in1.astype(np.float32)) * np.float32(c0)
        ).astype(np.float32),
    ),
)

# (a * b) * c0   (fold 1/den + EU scale into coefficients)
CMULS = _register(
    "BC6CMULS_ANT",
    Spec(
        body=(Src0 * Src1) * C0,
        reference=lambda in0, in1, c0, c1, c2: (
            (in0.astype(np.float32) * in1.astype(np.float32)) * np.float32(c0)
        ).astype(np.float32),
    ),
)

# (a + c0) * b * c1  (final decode fuse: (sdf + FLOOR_OFF_H) * e2 * K,
# K corrects the fp16 rounding of the ACT exp bias)
FMADD = _register(
    "BC6FMADDS_ANT",
    Spec(
        body=(Src0 + C0) * Src1 * C1,
        reference=lambda in0, in1, c0, c1, c2: (
            (in0.astype(np.float32) + np.float32(c0))
            * in1.astype(np.float32)
            * np.float32(c1)
        ).astype(np.float32),
    ),
)

# fp32 value the ACT bias tile holds, and the exact correction factor
# K = 2^-14 / exp(bias32) folded into the final fused multiply
_EXP_BIAS = float(np.float32(-14.0 * LN2))
_EXP_K = float(np.exp(-(14.0 * LN2 + _EXP_BIAS)))


# ------------------------------------------------------- bass kernel build
def _ap(base, dims):
    """Manual free-dim AP: keep base's partition dim, set free dims."""
    return bass.AP(base.tensor, base.offset, [list(base.ap[0])] + dims)


def build_kernel(nb_core=NB_CORE, g=G, eng_cfg=None):
    st_blocks = 128 * g
    n_st = nb_core // st_blocks
    assert nb_core % st_blocks == 0
    n_ch = (g * 32) // 128          # 128-col logit chunks per supertile
    assert n_ch % 4 == 0
    n_pt = n_ch // 4                # PSUM matmul tiles (4 chunks each)

    nc = bacc.Bacc(
        "TRN2",
        target_bir_lowering=False,
        debug=False,
        enable_asserts=False,
        num_devices=1,
    )

    # which engine runs each elementwise op (V=vector, G=gpsimd)
    cfg = dict(
        d02="V", d13="V", dd="V", bu="V", cur="V", dur="V",
        A1="V", A2="V", A3="V", A4="V", A5="V", A6="V",
        T1="V", T3="V", FM="V",
    )
    if eng_cfg:
        cfg.update(eng_cfg)

    ep = nc.dram_tensor("endpoints", [nb_core, 12], F16, kind="ExternalInput").ap()
    ix = nc.dram_tensor("indices", [nb_core, 16], F16, kind="ExternalInput").ap()
    lg = nc.dram_tensor("logits", [nb_core, 32], F16, kind="ExternalInput").ap()
    # bank_diag [128, 68]: row k (band q = k//32) has bank3[k%32, p] at col
    # 4p+q and 1.0 at col 64+q ("p=16"); zeros elsewhere.  One K=128 matmul
    # per transposed chunk yields num[(p,q)] + den interleaved, uniform
    # stride 4 in p, so the ACT PSUM->SBUF copy and all consumers use
    # affine APs.
    bank = nc.dram_tensor("bankd", [128, 68], F16, kind="ExternalInput").ap()
    ident = nc.dram_tensor("ident", [128, 128], F16, kind="ExternalInput").ap()
    out = nc.dram_tensor("out", [nb_core, 48], F32, kind="ExternalOutput").ap()

    with nc.allow_low_precision("fp16 kernel by design (validated 2.7e-3)"):
        with tile.TileContext(nc) as tc, ExitStack() as ctx:
            const_pool = ctx.enter_context(tc.tile_pool(name="const", bufs=1))
            in_pool = ctx.enter_context(tc.tile_pool(name="inp", bufs=4))
            sig_pool = ctx.enter_context(tc.tile_pool(name="sig", bufs=4))
            msk_pool = ctx.enter_context(tc.tile_pool(name="msk", bufs=3))
            et_pool = ctx.enter_context(tc.tile_pool(name="et", bufs=2))
            ta_pool = ctx.enter_context(tc.tile_pool(name="ta", bufs=2))
            cf_pool = ctx.enter_context(tc.tile_pool(name="cf", bufs=3))
            big_pool = ctx.enter_context(tc.tile_pool(name="big", bufs=3))
            dec_pool = ctx.enter_context(tc.tile_pool(name="dec", bufs=3))
            out_pool = ctx.enter_context(tc.tile_pool(name="outp", bufs=2))
            ps_t = ctx.enter_context(tc.tile_pool(name="ps_t", bufs=2, space="PSUM"))
            ps_mm = ctx.enter_context(tc.tile_pool(name="ps_mm", bufs=4, space="PSUM"))

            bank_t = const_pool.tile([128, 68], F16)
            nc.sync.dma_start(bank_t[:], bank)
            id_t = const_pool.tile([128, 128], F16)
            nc.sync.dma_start(id_t[:], ident)
            ebias_t = const_pool.tile([128, 1], F32)
            nc.gpsimd.memset(ebias_t[:], _EXP_BIAS)

            eng = {"V": nc.vector, "G": nc.gpsimd}

            def stage_in(t):
                """Loads, sigmoids, w, endpoint coeffs, mask path -> num_t."""
                b0 = t * st_blocks
                # ep and ix share one tile so a single ACT sigmoid covers both
                epix_t = in_pool.tile([128, g * 28], F16, tag="epix")
                ep_t = epix_t[:, 0 : g * 12]
                ix_t = epix_t[:, g * 12 : g * 28]
                lg_t = in_pool.tile([128, g * 32], F16, tag="lg")
                with tc.high_priority(offset=200):
                    nc.sync.dma_start(
                        ep_t,
                        ep[b0 : b0 + st_blocks, :].rearrange(
                            "(r g) d -> r (g d)", g=g
                        ),
                    )
                    nc.sync.dma_start(
                        ix_t,
                        ix[b0 : b0 + st_blocks, :].rearrange(
                            "(r g) d -> r (g d)", g=g
                        ),
                    )
                    nc.sync.dma_start(
                        lg_t[:],
                        lg[b0 : b0 + st_blocks, :].rearrange(
                            "(r g) d -> r (g d)", g=g
                        ),
                    )

                epix_s = sig_pool.tile([128, g * 28], F16, tag="epixs")
                # pull the sigmoid ahead of the rest of this iteration's ACT
                # work so w/d02 never wait on it
                with tc.high_priority(offset=140):
                    nc.scalar.activation(epix_s[:], epix_t[:], AFn.Sigmoid)

                s2u = cf_pool.tile([128, g * 3], F16, tag="s2u")
                nc.scalar.activation(
                    _ap(s2u[:], [[g, 3], [1, g]]),
                    _ap(epix_s[:, 6:9], [[1, 3], [12, g]]), AFn.Copy,
                    bias=float(EU_BIAS - FLOOR_OFF_H), scale=float(EU_SCALE),
                )

                # ---- logits: PE transpose -> ACT exp -> E_T (fp16) ----
                e_T = et_pool.tile([128, g * 32], F16, tag="eT")
                for j in range(0, n_ch, 8):
                    pst = ps_t.tile([128, 1024], F16, tag="pst")
                    for q in range(8):
                        ch = j + q
                        nc.tensor.transpose(
                            pst[:, 128 * q : 128 * (q + 1)],
                            lg_t[:, 128 * ch : 128 * (ch + 1)],
                            id_t[:],
                        )
                    nc.scalar.activation(
                        e_T[:, 128 * j : 128 * (j + 8)], pst[:], AFn.Exp
                    )

                # ---- per-chunk matmuls + ACT copies PSUM -> num_t ----
                # num_t [128, 17*g]: col p*g + gg, p=16 row holds den.
                num_t = msk_pool.tile([128, 17 * g], F16, tag="numt")
                for tp in range(n_pt):
                    pmm = ps_mm.tile([128, 272], F32, tag="pmm")
                    for chp in range(4):
                        ch = 4 * tp + chp
                        nc.tensor.matmul(
                            pmm[:, 68 * chp : 68 * (chp + 1)],
                            e_T[:, 128 * ch : 128 * (ch + 1)],
                            bank_t[:],
                            start=True,
                            stop=True,
                        )
                    nc.scalar.activation(
                        _ap(num_t[:, 16 * tp : 16 * tp + 4], [[4, 4], [1, 4], [g, 17]]),
                        _ap(pmm[:], [[68, 4], [1, 4], [4, 17]]),
                        AFn.Copy,
                    )

                return dict(b0=b0, epix_s=epix_s, num_t=num_t, s2u=s2u)

            def stage_in_back(c):
                """V/G input-side elementwise ops, emitted after the previous
                supertile's assembly so the vector queue never waits on this
                iteration's ACT sigmoid."""
                epix_s = c["epix_s"]

                def eslice(i):
                    return _ap(epix_s[:, 3 * i : 3 * i + 3], [[1, 3], [12, g]])

                # ---- w (custom DVE, (p,g)-transposing write) ----
                w_t = sig_pool.tile([128, g * 16], F16, tag="w")
                nc.vector._custom_dve(
                    BC6W,
                    out=w_t[:],
                    in0=_ap(epix_s[:, g * 12 : g * 12 + 16], [[1, 16], [16, g]]),
                    s0=7.0,
                    s1=-3.0,
                    imm2=1.0 / 64.0,
                )

                # ---- endpoint coefficients, (c, g) layout ----
                # d02 and d13 in one tensor_tensor: [2, c, g] tile, in0/in1
                # read sig(0|1) and sig(2|3) via a stride-3 leading dim
                d0213 = cf_pool.tile([128, g * 6], F16, tag="d0213")
                eng[cfg["d02"]].tensor_sub(
                    _ap(d0213[:], [[g * 3, 2], [g, 3], [1, g]]),
                    _ap(epix_s[:, 0:3], [[3, 2], [1, 3], [12, g]]),
                    _ap(epix_s[:, 6:9], [[3, 2], [1, 3], [12, g]]),
                )
                d02v = _ap(d0213[:], [[g, 3], [1, g]])
                d13v = _ap(d0213[:, g * 3 : g * 6], [[g, 3], [1, g]])
                dd = cf_pool.tile([128, g * 3], F16, tag="dd")
                ddv = _ap(dd[:], [[g, 3], [1, g]])
                eng[cfg["dd"]].tensor_sub(ddv, d13v, d02v)
                bu = cf_pool.tile([128, g * 3], F16, tag="bu")
                nc.vector._custom_dve(
                    CDIFFS,
                    out=_ap(bu[:], [[g, 3], [1, g]]),
                    in0=eslice(3),
                    in1=eslice(2),
                    s0=EU_SCALE,
                )
                # Bu*w: consumed by A5 only next iteration.
                w_b = _ap(w_t[:], [[0, 3], [g, 16], [1, g]])
                tB = big_pool.tile([128, g * 48], F16, tag="tB")
                eng[cfg["A4"]].tensor_mul(
                    tB[:], _ap(bu[:], [[g, 3], [0, 16], [1, g]]), w_b
                )
                c.update(w_t=w_t, d02v=d02v, ddv=ddv, tB=tB)

            def stage_asm(c):
                tB = c["tB"]
                """rcp-folded coefficients + assembly + round (V/G)."""
                rcp = cf_pool.tile([128, g], F16, tag="rcp")
                nc.vector.reciprocal(rcp[:], c["num_t"][:, 16 * g : 17 * g])
                rcp_b = _ap(rcp[:], [[0, 3], [1, g]])
                cur = cf_pool.tile([128, g * 3], F16, tag="cur")
                nc.vector._custom_dve(
                    CMULS, out=_ap(cur[:], [[g, 3], [1, g]]),
                    in0=c["d02v"], in1=rcp_b, s0=EU_SCALE,
                )
                dur = cf_pool.tile([128, g * 3], F16, tag="dur")
                nc.vector._custom_dve(
                    CMULS, out=_ap(dur[:], [[g, 3], [1, g]]),
                    in0=c["ddv"], in1=rcp_b, s0=EU_SCALE,
                )

                def cb(tile_):   # (c,g) coeff -> (c, p, g) broadcast
                    return _ap(tile_[:], [[g, 3], [0, 16], [1, g]])

                w_b = _ap(c["w_t"][:], [[0, 3], [g, 16], [1, g]])
                num_b = _ap(c["num_t"][:], [[0, 3], [g, 16], [1, g]])

                tA = ta_pool.tile([128, g * 48], F16, tag="tA")
                up = big_pool.tile([128, g * 48], F16, tag="up")
                hh = big_pool.tile([128, g * 48], F16, tag="hh")

                eng[cfg["A1"]].tensor_mul(tA[:], cb(dur), w_b)       # Dur*w
                eng[cfg["A2"]].tensor_add(tA[:], tA[:], cb(cur))     # +Cur
                eng[cfg["A3"]].tensor_mul(tA[:], tA[:], num_b)       # *num
                eng[cfg["A5"]].tensor_add(tA[:], tA[:], tB[:])       # +Bu*w
                eng[cfg["A6"]].tensor_add(up[:], tA[:], cb(c["s2u"]))  # = u'
                # hh = round(u') via fp32 magic add (DVE ALU is fp32)
                eng[cfg["T1"]].tensor_scalar(
                    hh[:], up[:], MAGIC, MAGIC, AOp.add, AOp.subtract
                )
                return dict(b0=c["b0"], up=up, hh=hh)

            def tail_sub(a):
                """sdf = u' - hh; deps are one iteration old."""
                sdf = dec_pool.tile([128, g * 48], F16, tag="sdf")
                eng[cfg["T3"]].tensor_sub(sdf[:], a["up"][:], a["hh"][:])
                return sdf

            def tail_exp(a):
                """ACT exp of hh, emitted last in the iteration so the ACT
                queue serves this iteration's sigmoid first (e2 is consumed
                only next iteration)."""
                e2 = dec_pool.tile([128, g * 48], F16, tag="e2")
                nc.scalar.activation(
                    e2[:], a["hh"][:], AFn.Exp, bias=ebias_t[:], scale=LN2
                )
                return e2

            def tail_store(a, e2, sdf):
                o_t = out_pool.tile([128, g * 48], F32, tag="o")
                # out is (g, c, p) block-major for contiguous DMA; the fused
                # op reads (c, p, g) streams ((c,p) collapses to one uniform
                # dim) and scatters on write (1x op).
                nc.vector._custom_dve(
                    FMADD,
                    out=_ap(o_t[:], [[1, 48], [48, g]]),
                    in0=_ap(sdf[:], [[g, 48], [1, g]]),
                    in1=_ap(e2[:], [[g, 48], [1, g]]),
                    s0=FLOOR_OFF_H,
                    s1=_EXP_K,
                )
                nc.sync.dma_start(
                    out[a["b0"] : a["b0"] + st_blocks, :].rearrange(
                        "(r g) d -> r (g d)", g=g
                    ),
                    o_t[:],
                )

            # 3-stage software pipeline: mask path of tile t runs alongside
            # the assembly of t-1, the exp/subtract of t-2, and the final
            # fuse+store of t-3, so no engine queue ever waits on a
            # same-iteration cross-engine product (gpsimd's slow T3 gets a
            # full iteration before FM consumes it).
            in_ctxs = {}
            asm_outs = {}
            sub_outs = {}
            exp_outs = {}
            for t in range(n_st + 3):
                if 2 <= t < n_st + 2:
                    sub_outs[t - 2] = tail_sub(asm_outs[t - 2])
                if t < n_st:
                    in_ctxs[t] = stage_in(t)
                if 1 <= t <= n_st:
                    asm_outs[t - 1] = stage_asm(in_ctxs.pop(t - 1))
                if t < n_st:
                    with tc.high_priority(offset=-15):
                        stage_in_back(in_ctxs[t])
                if 2 <= t < n_st + 2:
                    exp_outs[t - 2] = tail_exp(asm_outs[t - 2])
                if t >= 3:
                    tail_store(
                        asm_outs.pop(t - 3),
                        exp_outs.pop(t - 3),
                        sub_outs.pop(t - 3),
                    )

    nc.compile()
    return nc


# ------------------------------------------------------- host-side driver
_NC_CACHE = {}


def _get_nc():
    if "nc" not in _NC_CACHE:
        _NC_CACHE["nc"] = build_kernel()
    return _NC_CACHE["nc"]


def make_in_maps(endpoints, indices, partition_logits, partition_bank, nb=NB):
    """Shard + pack host inputs into the 8 per-core input dicts."""
    bank16 = partition_bank.astype(np.float16)       # 0/1 exact
    bankd = np.zeros((128, 68), dtype=np.float16)
    for q in range(4):
        rows = slice(32 * q, 32 * (q + 1))
        bankd[rows, q : 64 + q : 4] = bank16         # col 4p+q, p=0..15
        bankd[rows, 64 + q] = 1.0                    # den
    ident = np.eye(128, dtype=np.float16)

    ep_flat = endpoints.reshape(nb, 12).astype(np.float16)
    ixf = indices.astype(np.float16)
    lgf = partition_logits.astype(np.float16)
    nbc = nb // N_CORES
    in_maps = []
    for c in range(N_CORES):
        sl = slice(c * nbc, (c + 1) * nbc)
        in_maps.append(
            {
                "endpoints": np.ascontiguousarray(ep_flat[sl]),
                "indices": np.ascontiguousarray(ixf[sl]),
                "logits": np.ascontiguousarray(lgf[sl]),
                "bankd": bankd,
                "ident": ident,
            }
        )
    return in_maps


def blocks_to_img(blocks):
    """[NB, 48] c-major blocks -> (3, H, W) image."""
    return (
        blocks.reshape(BY, BX, 3, 4, 4)
        .transpose(2, 0, 3, 1, 4)
        .reshape(3, H, W)
        .astype(np.float32)
    )


def kernel(endpoints, indices, partition_logits, partition_bank, weight_lut):
    endpoints = np.asarray(endpoints, dtype=np.float32)
    indices = np.asarray(indices, dtype=np.float32)
    partition_logits = np.asarray(partition_logits, dtype=np.float32)
    partition_bank = np.asarray(partition_bank, dtype=np.float32)
    assert endpoints.shape[0] == NB

    in_maps = make_in_maps(endpoints, indices, partition_logits, partition_bank)
    nc = _get_nc()
    res = bass_utils.run_bass_kernel_spmd(
        nc, in_maps, core_ids=list(range(N_CORES))
    )
    blocks = np.concatenate(
        [res.results[c]["out"] for c in range(N_CORES)], axis=0
    )
    return blocks_to_img(blocks)
